# Initial kernel scaffold
#
"""MergedEmbeddingBag kernel for 8 TRN2 NeuronCores.

Strategy (batch-sharded SPMD):
  - Global work: T=26 tables x B=4096 bags of L=10 lookups each into
    [V=50000, D=128] f32 tables, pooled with sum, concat with dense.
  - The T*B = 106496 bags (t-major order) are split into 8 contiguous
    chunks of 13312 bags; core m handles chunk m.  A chunk spans at most
    4 consecutive tables, so each core gets a 4-table weight window
    (200000 rows) and bag indices pre-biased on the host to be rows into
    that window.  Every core runs the identical program (pure SPMD).
  - Per core: 13 calls x [128 partitions x 80 rows] indirect-DMA gather
    (10240 rows of 512B per call), then an in-place DVE add tree pools
    the L=10 rows of each bag, then one strided store of 1024 pooled
    bags.  The dense column block is passed through on the host.
"""

import numpy as np

import concourse.bacc as bacc
import concourse.bass as bass
import concourse.mybir as mybir
import concourse.tile as tile
from concourse.bass_utils import run_bass_kernel_spmd

T, B, L, V, D = 26, 4096, 10, 50000, 128
M = 8                          # cores
BPC = T * B // M               # 13312 bags per core
PART = 128                     # SBUF partitions
BAGS_PER_PART = 8              # bags pooled per partition per call
BAGS_PER_CALL = PART * BAGS_PER_PART   # 1024
CALLS = BPC // BAGS_PER_CALL   # 13
K = BAGS_PER_PART * L          # 80 gathered rows per partition per call
WT = 4                         # tables in each core's weight window
W_ROWS = WT * V                # 200000
# first table of each core's window ((m*BPC)//B, all chunks fit in 4 tables)
TBASE = [(m * BPC) // B for m in range(M)]

_CACHE = {}


def _build_nc():
    if "nc" in _CACHE:
        return _CACHE["nc"]
    nc = bacc.Bacc("TRN2", target_bir_lowering=False, debug=False, num_devices=M)
    w = nc.dram_tensor("w", [W_ROWS, D], mybir.dt.float32, kind="ExternalInput").ap()
    idx = nc.dram_tensor(
        "idx", [PART, CALLS * K], mybir.dt.int32, kind="ExternalInput"
    ).ap()
    out = nc.dram_tensor(
        "out", [BPC, D], mybir.dt.float32, kind="ExternalOutput"
    ).ap()
    # out row (c*1024 + p*8 + j) <- pooled[p, j*128:(j+1)*128] of call c
    out_v = out.rearrange("(c p j) d -> c p (j d)", c=CALLS, p=PART, j=BAGS_PER_PART)

    J = BAGS_PER_PART
    BLK = J * D  # 1024 elems = one l-block (8 bags x 128)

    with tile.TileContext(nc) as tc:
        with (
            tc.tile_pool(name="idxp", bufs=1) as idxp,
            tc.tile_pool(name="gathp", bufs=3) as gathp,
        ):
            idx_sb = idxp.tile([PART, CALLS * K], mybir.dt.int32)
            nc.sync.dma_start(out=idx_sb[:], in_=idx[:])
            for c in range(CALLS):
                gath = gathp.tile([PART, K * D], mybir.dt.float32)
                # column k = l*8 + j of the idx slice gathers the l-th row of
                # bag (c*1024 + p*8 + j) into gath[p, k*128:(k+1)*128]
                nc.gpsimd.indirect_dma_start(
                    out=gath[:].rearrange("p (k d) -> p k d", d=D),
                    out_offset=None,
                    in_=w[:],
                    in_offset=bass.IndirectOffsetOnAxis(
                        ap=idx_sb[:, c * K : (c + 1) * K], axis=0
                    ),
                )
                # in-place add tree over the 10 l-blocks of BLK elems each
                nc.vector.tensor_add(
                    out=gath[:, : 5 * BLK],
                    in0=gath[:, : 5 * BLK],
                    in1=gath[:, 5 * BLK : 10 * BLK],
                )
                nc.vector.tensor_add(
                    out=gath[:, : 2 * BLK],
                    in0=gath[:, : 2 * BLK],
                    in1=gath[:, 2 * BLK : 4 * BLK],
                )
                nc.vector.tensor_add(
                    out=gath[:, :BLK], in0=gath[:, :BLK], in1=gath[:, BLK : 2 * BLK]
                )
                nc.vector.tensor_add(
                    out=gath[:, :BLK], in0=gath[:, :BLK], in1=gath[:, 4 * BLK : 5 * BLK]
                )
                nc.sync.dma_start(out=out_v[c], in_=gath[:, :BLK])
    nc.compile()
    _CACHE["nc"] = nc
    return nc


def _prep_inputs(index, weights):
    """Per-core input maps: 4-table weight window + rearranged local indices."""
    index = np.asarray(index, dtype=np.int32)
    gidx = index + (np.arange(T, dtype=np.int32) * V)[:, None]  # global rows
    bags = gidx.reshape(T * B, L)
    w_flat = np.asarray(weights, dtype=np.float32).reshape(T * V, D)
    in_maps = []
    for m in range(M):
        tb = TBASE[m]
        loc = bags[m * BPC : (m + 1) * BPC] - np.int32(tb * V)  # [13312, 10]
        # bag i = c*1024 + p*8 + j ; sbuf col = c*80 + l*8 + j
        arr = (
            loc.reshape(CALLS, PART, BAGS_PER_PART, L)
            .transpose(0, 1, 3, 2)  # [c, p, l, j]
            .reshape(CALLS, PART, K)
            .transpose(1, 0, 2)  # [p, c, K]
            .reshape(PART, CALLS * K)
        )
        in_maps.append(
            {
                "w": w_flat[tb * V : (tb + WT) * V],
                "idx": np.ascontiguousarray(arr),
            }
        )
    return in_maps


def kernel(index, offsets, dense, weights):
    nc = _build_nc()
    in_maps = _prep_inputs(index, weights)
    res = run_bass_kernel_spmd(nc, in_maps, core_ids=list(range(M))).results
    pooled = np.concatenate([res[m]["out"] for m in range(M)], axis=0)
    pooled = pooled.reshape(T, B, D)
    out = np.empty((B, (T + 1) * D), np.float32)
    out[:, :D] = np.asarray(dense, dtype=np.float32)
    out[:, D:] = pooled.transpose(1, 0, 2).reshape(B, T * D)
    return out


# revision 6
# speedup vs baseline: 3.5619x; 3.5619x over previous
"""MergedEmbeddingBag kernel for 8 TRN2 NeuronCores.

Strategy (batch-sharded SPMD + per-table-pair compaction + dma_gather):
  - Global work: T=26 tables x B=4096 bags of L=10 lookups each into
    [V=50000, D=128] f32 tables, sum-pooled, concat with dense.
  - Batch sharding: core m handles bags [m*512, (m+1)*512) of EVERY
    table -> 26*512 = 13312 bags/core, perfectly uniform SPMD.
  - The fast gather path is the Q7 `dma_gather` extended instruction
    (vectorized descriptor generation), whose indices are int16.  To fit
    int16, the host compacts weights per (core, table-pair): the <=10240
    distinct rows referenced by one core in tables (2s, 2s+1) are packed
    into slot s of a [13*10240, 128] per-core weight buffer, and the
    lookup indices are remapped to compacted ids (< 10240).
  - Per core: 13 dma_gather calls of 10240 rows (one per table pair),
    in-place DVE add tree pools the L=10 rows of each bag, one strided
    store per call.  The dense column block is passed through on host.

dma_gather HW contract (probed on silicon):
  - stream position i reads its int16 index from idxs tile partition
    16 + (i%16), word i//16 (queue 0).  (The CoreSim reads partitions
    0..15, so indices are duplicated into both ranges.)
  - gathered row i lands in dst partition i%128, free slot i//128.
"""

import numpy as np

import concourse.bacc as bacc
import concourse.bass as bass
import concourse.mybir as mybir
import concourse.tile as tile
from concourse.bass_utils import run_bass_kernel_spmd

T, B, L, V, D = 26, 4096, 10, 50000, 128
M = 8                          # cores
BPC = T * B // M               # 13312 bags per core
BAGS_PER_TABLE = B // M        # 512
PAIRS = T // 2                 # 13 table pairs == calls per core
BAGS_PER_CALL = 2 * BAGS_PER_TABLE  # 1024
NIDX = BAGS_PER_CALL * L       # 10240 gathered rows per call
CAP = NIDX                     # compacted rows capacity per pair slot
W_ROWS = PAIRS * CAP           # 133120
IDXW = NIDX // 16              # 640 idx words per channel per call

_CACHE = {}


def _build_nc(repeats=1):
    key = ("nc", repeats)
    if key in _CACHE:
        return _CACHE[key]
    nc = bacc.Bacc("TRN2", target_bir_lowering=False, debug=False, num_devices=M)
    w = nc.dram_tensor("w", [W_ROWS, D], mybir.dt.float32, kind="ExternalInput").ap()
    idx = nc.dram_tensor(
        "idx", [128, PAIRS * IDXW], mybir.dt.int16, kind="ExternalInput"
    ).ap()
    out = nc.dram_tensor("out", [BPC, D], mybir.dt.float32, kind="ExternalOutput").ap()
    # out row (c*1024 + p*8 + j) <- pooled[p, j*128:(j+1)*128] of call c
    out_v = out.rearrange("(c p j) d -> c p (j d)", c=PAIRS, p=128, j=8)

    BLK = 8 * D  # 1024 elems = one l-block (8 bags x 128)

    with tile.TileContext(nc) as tc:
        with (
            tc.tile_pool(name="idxp", bufs=1) as idxp,
            tc.tile_pool(name="gathp", bufs=3) as gathp,
        ):
            idx_sb = idxp.tile([128, PAIRS * IDXW], mybir.dt.int16)
            nc.sync.dma_start(out=idx_sb[:], in_=idx[:])
            for c in [c for _ in range(repeats) for c in range(PAIRS)]:
                gath = gathp.tile([128, NIDX], mybir.dt.float32)
                nc.gpsimd.dma_gather(
                    out_ap=gath[:].rearrange("p (k d) -> p k d", d=D),
                    in_ap=w[c * CAP : (c + 1) * CAP, :],
                    idxs_ap=idx_sb[:, c * IDXW : (c + 1) * IDXW],
                    num_idxs=NIDX,
                    num_idxs_reg=NIDX,
                    elem_size=D,
                    single_packet=False,
                )
                # in-place add tree over the 10 l-blocks of BLK elems each
                nc.vector.tensor_add(
                    out=gath[:, : 5 * BLK],
                    in0=gath[:, : 5 * BLK],
                    in1=gath[:, 5 * BLK : 10 * BLK],
                )
                nc.vector.tensor_add(
                    out=gath[:, : 2 * BLK],
                    in0=gath[:, : 2 * BLK],
                    in1=gath[:, 2 * BLK : 4 * BLK],
                )
                nc.vector.tensor_add(
                    out=gath[:, :BLK], in0=gath[:, :BLK], in1=gath[:, BLK : 2 * BLK]
                )
                nc.vector.tensor_add(
                    out=gath[:, :BLK], in0=gath[:, :BLK], in1=gath[:, 4 * BLK : 5 * BLK]
                )
                nc.sync.dma_start(out=out_v[c], in_=gath[:, :BLK])
    nc.compile()
    _CACHE[key] = nc
    return nc


def _prep_inputs(index, weights):
    """Per-core inputs: compacted pair-wise weights + snake-laid int16 ids."""
    index = np.asarray(index)
    w_flat = np.asarray(weights, dtype=np.float32).reshape(T * V, D)
    in_maps = []
    for m in range(M):
        # per-table slice of this core's 512 bags -> [T, 5120]
        idx_m = index[:, m * BAGS_PER_TABLE * L : (m + 1) * BAGS_PER_TABLE * L]
        w_core = np.zeros((W_ROWS, D), np.float32)
        idx_core = np.zeros((128, PAIRS * IDXW), np.int16)
        for s in range(PAIRS):
            # local row key within the pair: [0, 2V)
            keys = np.concatenate(
                [idx_m[2 * s], idx_m[2 * s + 1] + V]
            )  # [10240] order: table 2s bags, then 2s+1 bags
            uniq, inv = np.unique(keys, return_inverse=True)
            u = len(uniq)
            assert u <= CAP
            w_core[s * CAP : s * CAP + u] = w_flat[2 * s * V + uniq]
            # arr[q, l]: compact id of element l of call-local bag q
            arr = inv.reshape(BAGS_PER_CALL, L)
            # stream position i = (l*8+j)*128 + p for bag q = p*8+j
            lst = (
                arr.reshape(128, 8, L).transpose(2, 1, 0).reshape(NIDX).astype(np.int16)
            )
            # snake: stream[i] read from partition 16+(i%16) (HW) / (i%16) (sim)
            snake = lst.reshape(IDXW, 16).T  # [16, IDXW]
            idx_core[0:16, s * IDXW : (s + 1) * IDXW] = snake
            idx_core[16:32, s * IDXW : (s + 1) * IDXW] = snake
        in_maps.append({"w": w_core, "idx": idx_core})
    return in_maps


def kernel(index, offsets, dense, weights):
    nc = _build_nc()
    in_maps = _prep_inputs(index, weights)
    res = run_bass_kernel_spmd(nc, in_maps, core_ids=list(range(M))).results
    # res[m]["out"][i_loc] = pooled(t=i_loc//512, b=m*512 + i_loc%512)
    pooled = np.empty((T, B, D), np.float32)
    for m in range(M):
        pooled[:, m * BAGS_PER_TABLE : (m + 1) * BAGS_PER_TABLE] = res[m][
            "out"
        ].reshape(T, BAGS_PER_TABLE, D)
    out = np.empty((B, (T + 1) * D), np.float32)
    out[:, :D] = np.asarray(dense, dtype=np.float32)
    out[:, D:] = pooled.transpose(1, 0, 2).reshape(B, T * D)
    return out
